# revision 4
# baseline (speedup 1.0000x reference)
"""Trainium2 Bass kernel for nn_ColOutlierLinear.

Computes out = f16(x16 @ dequant(qweight).T) + f16(x16[:, outlier_idx] @ W_fp16.T)
              + f16(bias)   (fp16, staged rounding matching the jax reference)

Strategy (tensor-parallel over output dim N across 8 cores):
  - Host: dequantize qweight exactly as the reference does (stepwise fp16
    math: w = f16(f16(sign(q/127)*(q/127)^2) * f16(scale))), transpose to
    [K, N], shard columns across 8 cores, and pack everything into a
    partition-major layout so every device DMA is contiguous per partition.
  - Device (per core): two fp32-PSUM GEMM accumulations — 63 normal
    k-chunks into psA and one outlier k-chunk into psB — then an epilogue
    replicating the reference's f16 staging: f16(psA) + f16(psB) + f16(bias).
    Weights stream chunk-wise, double-buffered; x stays resident in SBUF.
  - Weights travel as WDTYPE (float16 exact, or float8_e4m3 at 1 byte/elt),
    pre-scaled by ALPHA=16 on host (exact power-of-two shift) so fp8 never
    underflows; the PSUM->SBUF cast applies 1/ALPHA for free.
"""

import sys

if "/opt/trn_rl_repo" not in sys.path:
    sys.path.insert(0, "/opt/trn_rl_repo")

import numpy as np
import ml_dtypes

import concourse.bass as bass
import concourse.tile as tile
from concourse import bacc, mybir
from concourse.bass_utils import run_bass_kernel_spmd

# ---- problem geometry (hardcoded per the harness contract) ----
B = 64          # batch rows
N = 8192        # output dim
KN = 8064       # normal (non-outlier) columns
KO = 128        # outlier columns
BLOCK = 64      # quantization block size
NCORES = 8
N_C = N // NCORES          # 1024 output cols per core
NCH = KN // 128            # 63 normal k-chunks of 128
ALPHA = 16.0               # power-of-two weight pre-scale (undone on PSUM copy)

WDTYPE = "f16"             # "f16" (exact) or "f8" (float8_e4m3 weights)
G = 3                      # k-chunks per weight DMA (63 = 21 * 3)
WBUFS = 6                  # weight pool buffer depth

_DT = {"f16": mybir.dt.float16, "f8": mybir.dt.float8e4}
_NPDT = {"f16": np.float16, "f8": ml_dtypes.float8_e4m3}


def _build(wdtype_key):
    wdt = _DT[wdtype_key]
    f16 = mybir.dt.float16
    f32 = mybir.dt.float32

    nc = bacc.Bacc("TRN2", target_bir_lowering=False, debug=False)
    wq = nc.declare_dram_parameter("wq", [128, NCH * N_C], wdt, isOutput=False)
    wo = nc.declare_dram_parameter("wo", [128, N_C], f16, isOutput=False)
    xn = nc.declare_dram_parameter("xn", [128, NCH * B], f16, isOutput=False)
    xo = nc.declare_dram_parameter("xo", [128, B], f16, isOutput=False)
    bb = nc.declare_dram_parameter("bb", [B, N_C], f16, isOutput=False)
    out = nc.declare_dram_parameter("out", [B, N_C], f16, isOutput=True)

    with tile.TileContext(nc) as tc:
        with (
            tc.tile_pool(name="xpool", bufs=1) as xpool,
            tc.tile_pool(name="wpool", bufs=WBUFS) as wpool,
            tc.tile_pool(name="opool", bufs=1) as opool,
            tc.tile_pool(name="psum", bufs=1, space="PSUM") as pp,
        ):
            xt = xpool.tile([128, NCH * B], f16)
            nc.sync.dma_start(xt[:], xn[:])
            xot = xpool.tile([128, B], f16)
            nc.sync.dma_start(xot[:], xo[:])
            bt = xpool.tile([B, N_C], f16)
            nc.sync.dma_start(bt[:], bb[:])
            wot = xpool.tile([128, N_C], f16)
            nc.sync.dma_start(wot[:], wo[:])

            psA = pp.tile([B, N_C], f32)
            psB = pp.tile([B, N_C], f32)

            n_groups = NCH // G
            assert n_groups * G == NCH
            for g in range(n_groups):
                wt = wpool.tile([128, G * N_C], wdt)
                nc.sync.dma_start(wt[:], wq[:, g * G * N_C:(g + 1) * G * N_C])
                for j in range(G):
                    c = g * G + j
                    for h in range(2):
                        nc.tensor.matmul(
                            psA[:, h * 512:(h + 1) * 512],
                            xt[:, c * B:(c + 1) * B],
                            wt[:, j * N_C + h * 512: j * N_C + (h + 1) * 512],
                            start=(c == 0),
                            stop=(c == NCH - 1),
                        )
            for h in range(2):
                nc.tensor.matmul(
                    psB[:, h * 512:(h + 1) * 512],
                    xot[:],
                    wot[:, h * 512:(h + 1) * 512],
                    start=True,
                    stop=True,
                )

            # epilogue: replicate reference staging
            #   out = f16(f16(main) + f16(outlier_mm)) + f16(bias)
            cA = opool.tile([B, N_C], f16)
            nc.scalar.mul(cA[:, 0:512], psA[:, 0:512], 1.0 / ALPHA)
            nc.vector.tensor_scalar_mul(cA[:, 512:1024], psA[:, 512:1024], 1.0 / ALPHA)
            cB = opool.tile([B, N_C], f16)
            nc.scalar.mul(cB[:, 0:512], psB[:, 0:512], 1.0 / ALPHA)
            nc.vector.tensor_scalar_mul(cB[:, 512:1024], psB[:, 512:1024], 1.0 / ALPHA)
            t = opool.tile([B, N_C], f16)
            nc.vector.tensor_add(t[:], cA[:], cB[:])
            ot = opool.tile([B, N_C], f16)
            nc.vector.tensor_add(ot[:], t[:], bt[:])
            nc.sync.dma_start(out[:], ot[:])

    nc.compile()
    return nc


_CACHE = {}


def _get_nc():
    if WDTYPE not in _CACHE:
        _CACHE[WDTYPE] = _build(WDTYPE)
    return _CACHE[WDTYPE]


def _pack(a, nchunks, width):
    """[nchunks*128, width] row-major -> [128, nchunks*width] partition-major."""
    return np.ascontiguousarray(
        a.reshape(nchunks, 128, width).swapaxes(0, 1).reshape(128, nchunks * width)
    )


def _prepare_in_maps(x, qweight, scales, W_fp16, bias, normal_idx, outlier_idx):
    x = np.asarray(x)
    qweight = np.asarray(qweight)
    scales = np.asarray(scales)
    W_fp16 = np.asarray(W_fp16)
    bias = np.asarray(bias)
    normal_idx = np.asarray(normal_idx)
    outlier_idx = np.asarray(outlier_idx)

    n, k_pad = qweight.shape
    nb = k_pad // BLOCK
    assert (n, k_pad) == (N, KN) and x.shape == (B, N)

    # --- dequantize exactly like the reference (stepwise fp16 rounding) ---
    q16 = qweight.astype(np.float16)
    wc = (q16 / np.float16(127.0)).astype(np.float16)
    wn = (np.sign(wc) * wc * wc).astype(np.float16)
    s16 = scales.astype(np.float16)
    w16 = (wn.reshape(n, nb, BLOCK) * s16[:, :, None]).astype(np.float16)
    w16 = w16.reshape(n, k_pad)

    a16 = np.float16(ALPHA)
    wT = (w16.T * a16).astype(np.float16)                          # [KN, N]
    woT = (W_fp16.astype(np.float16).T * a16).astype(np.float16)   # [KO, N]
    bias16 = bias.astype(np.float16)                               # [N] (unscaled)

    x16 = x.astype(np.float16)
    xnT = np.ascontiguousarray(x16[:, normal_idx].T)               # [KN, B]
    xoT = np.ascontiguousarray(x16[:, outlier_idx].T)              # [KO, B]

    npdt = _NPDT[WDTYPE]
    in_maps = []
    for c in range(NCORES):
        cols = slice(c * N_C, (c + 1) * N_C)
        in_maps.append({
            "wq": _pack(wT[:, cols].astype(npdt), NCH, N_C),
            "wo": np.ascontiguousarray(woT[:, cols]),
            "xn": _pack(xnT, NCH, B),
            "xo": xoT,
            "bb": np.ascontiguousarray(
                np.broadcast_to(bias16[cols][None, :], (B, N_C))
            ),
        })
    return in_maps


def kernel(x, qweight, scales, W_fp16, bias, normal_idx, outlier_idx):
    in_maps = _prepare_in_maps(
        x, qweight, scales, W_fp16, bias, normal_idx, outlier_idx
    )
    nc = _get_nc()
    res = run_bass_kernel_spmd(nc, in_maps, list(range(NCORES)))
    out = np.concatenate([res.results[c]["out"] for c in range(NCORES)], axis=1)
    return out.astype(np.float16)


def run_traced(**inputs):
    """Test-only helper: run with NTFF profiling, return BassKernelResults."""
    in_maps = _prepare_in_maps(**inputs)
    nc = _get_nc()
    return run_bass_kernel_spmd(nc, in_maps, list(range(NCORES)), trace=True)
